# revision 55
# baseline (speedup 1.0000x reference)
"""AttentionPairBias kernel for 8 Trainium2 NeuronCores.

Sharding: rows of the query sequence (S=1024) are split across the 8 cores
(128 rows each). The pair tensor z's bias contribution, the softmax and the
output rows are all embarrassingly parallel in the query dimension, so no
collectives are needed; each core reads its own 128x1024x128 slice of z.

Per-core pipeline (stage B projections are interleaved with stage C groups
so early pair-bias groups fill the PE while the weights stream in):
  1. z arrives pre-transposed from the host as zT [c=128, row, t] (bf16), so
     each 4-row group is one large straight DMA [128, 4096] (contiguous 8KB
     per partition) instead of a serialized XBAR transpose, alternating
     between the two HWDGE rings.
  2. DVE squares zT; per row j (PE col strip j) one matmul with the mean-
     centered augmented weight [.. | ln_w*Wz^T - c1/DZ | ones/DZ] -> py bank
     and one on zT^2 with ones/DZ replicated into all 32 cols -> pss bank.
     All matmuls are single-shot fp32 N=512 in separate banks: the bank-wide
     has_written clear makes concurrent accumulation chains unsafe, while
     single-shot groups let all four col strips stream concurrently.
  3. ACT evacuates both banks full-width (a DVE op cannot read two PSUM
     operands; partition-sliced copies are lane-starved); y/mean/E[z^2]
     round-trip through DRAM on the SWDGE queue (HWDGE rings would stall the
     z stream behind evacuation sem-waits) to re-slice [ch,t]-per-row into
     [row,t]-per-channel tiles.
  4. r = rsqrt(var+eps) via Ln/Exp; bias_h = r*y'_h directly - the c1*mu
     centering is folded into W18's columns (mu is itself a z-contraction),
     and the ln_b term is dropped (constant along t, softmax-invariant).
  5. Per head: scores = qk/sqrt(hd) + r*y' -> PE transpose -> exp on ACT
     (max-subtraction-free: |scores| < 4) -> A@[V|1] gives o and the softmax
     denominator in one accumulation chain.
  6. sigmoid gate, output projection.
"""

import os
import sys
import types
import numpy as np

for _p in ("/opt/trn_rl_repo", "/root/.axon_site/_ro/trn_rl_repo"):
    if os.path.isdir(_p) and _p not in sys.path:
        sys.path.append(_p)

import ml_dtypes
from contextlib import ExitStack

import concourse.bass as bass
import concourse.mybir as mybir
import concourse.tile as tile
from concourse import bacc
from concourse.bass import ds, ts
from concourse.masks import make_identity

BF16 = mybir.dt.bfloat16
FP32 = mybir.dt.float32
AF = mybir.ActivationFunctionType
ALU = mybir.AluOpType

S = 1024
D = 768
H = 16
HD = 48
HDP = 64            # padded head dim (2 heads per 128-partition block)
DP = H * HDP        # 1024
DZ = 128
EPS = 1e-5
N_CORES = 8
RPC = S // N_CORES  # 128 rows per core
ISQ = float(HD) ** -0.5

_CACHE = {}


def _build(c1):
    """Build the per-core SPMD program. c1[h] = sum_c ln_w[c]*Wz[h,c] are
    baked as immediates."""
    nc = bacc.Bacc("TRN2", target_bir_lowering=False, debug=False,
                   num_devices=N_CORES)

    zbT = nc.dram_tensor("zbT", [DZ, RPC, S], BF16, kind="ExternalInput").ap()
    sT = nc.dram_tensor("sT", [D, S], BF16, kind="ExternalInput").ap()
    sTc = nc.dram_tensor("sTc", [D, RPC], BF16, kind="ExternalInput").ap()
    WqT = nc.dram_tensor("WqT", [D, DP], BF16, kind="ExternalInput").ap()
    WkT = nc.dram_tensor("WkT", [D, DP], BF16, kind="ExternalInput").ap()
    WvT = nc.dram_tensor("WvT", [D, DP], BF16, kind="ExternalInput").ap()
    WgT = nc.dram_tensor("WgT", [D, D], BF16, kind="ExternalInput").ap()
    WoT = nc.dram_tensor("WoT", [D, D], BF16, kind="ExternalInput").ap()
    bqs = nc.dram_tensor("bqs", [DP], FP32, kind="ExternalInput").ap()
    W18 = nc.dram_tensor("W18", [DZ, 32], BF16, kind="ExternalInput").ap()
    Wss = nc.dram_tensor("Wss", [DZ, 32], BF16, kind="ExternalInput").ap()
    out = nc.dram_tensor("out", [RPC, D], FP32, kind="ExternalOutput").ap()

    with tile.TileContext(nc) as tc, ExitStack() as ctx:
        consts = ctx.enter_context(tc.tile_pool(name="consts", bufs=1))
        dram = ctx.enter_context(tc.tile_pool(name="dram", bufs=1, space="DRAM"))

        # All const loads go on the SWDGE (gpsimd) ring so the two HWDGE
        # rings (sync/scalar) stay dedicated to streaming z. Small stage-C
        # weights first so pair-bias can start immediately.
        w18_sb = consts.tile([128, 32], BF16, name="w18_sb")
        nc.gpsimd.dma_start(w18_sb[:], W18[:])
        wss_sb = consts.tile([128, 32], BF16, name="wss_sb")
        nc.gpsimd.dma_start(wss_sb[:], Wss[:])
        bq_sb = consts.tile([128, 8], FP32, name="bq_sb")
        nc.gpsimd.dma_start(bq_sb[:], bqs.rearrange("(b p) -> p b", p=128))
        sTc_sb = consts.tile([128, 6, RPC], BF16, name="sTc_sb")
        nc.gpsimd.dma_start(sTc_sb[:], sTc.rearrange("(a p) n -> p a n", p=128))
        sT_sb = consts.tile([128, 6, S], BF16, name="sT_sb")
        nc.gpsimd.dma_start(sT_sb[:], sT.rearrange("(a p) n -> p a n", p=128))
        wq_sb = consts.tile([128, 6, DP], BF16, name="wq_sb")
        nc.gpsimd.dma_start(wq_sb[:], WqT.rearrange("(a p) n -> p a n", p=128))
        wk_sb = consts.tile([128, 6, DP], BF16, name="wk_sb")
        nc.gpsimd.dma_start(wk_sb[:], WkT.rearrange("(a p) n -> p a n", p=128))
        wv_sb = consts.tile([128, 6, DP], BF16, name="wv_sb")
        nc.gpsimd.dma_start(wv_sb[:], WvT.rearrange("(a p) n -> p a n", p=128))
        wg_sb = consts.tile([128, 6, D], BF16, name="wg_sb")
        nc.gpsimd.dma_start(wg_sb[:], WgT.rearrange("(a p) n -> p a n", p=128))
        wo_sb = consts.tile([128, 6, D], BF16, name="wo_sb")
        nc.gpsimd.dma_start(wo_sb[:], WoT.rearrange("(a p) n -> p a n", p=128))
        ident = consts.tile([128, 128], BF16, name="ident")
        make_identity(nc, ident[:])
        eps_sb = consts.tile([128, 1], FP32, name="eps_sb")
        nc.vector.memset(eps_sb[:], EPS)

        kT_sb = consts.tile([128, 8, S], BF16, name="kT_sb")
        v_sb = consts.tile([128, 8, H, HDP + 1], BF16, name="v_sb")
        qT_sb = consts.tile([128, 8, RPC], BF16, name="qT_sb")
        g_sb = consts.tile([128, D], BF16, name="g_sb")
        oall = consts.tile([128, D], BF16, name="oall")
        mu_sb = consts.tile([128, S], BF16, name="mu_sb")
        ez2_sb = consts.tile([128, S], BF16, name="ez2_sb")
        r_sb = consts.tile([128, S], BF16, name="r_sb")
        var_sb = consts.tile([128, S], FP32, name="var_sb")

        # y round-trips through DRAM to re-slice [ch, t]-per-row into
        # [row, t]-per-channel: [row, c, t], c: 0 = E[z^2], 1..16 heads,
        # 17 = mean
        y_dram = dram.tile([RPC, 18, S], BF16)

        nc.vector.memset(v_sb[:, :, :, HDP:HDP + 1], 1.0)

        # ---- stage B (projections) + stage C (pair-bias) share pools so
        # the scheduler can overlap z transposes with projection matmuls ----
        with tc.tile_pool(name="psA", bufs=2, space="PSUM") as psA, \
             tc.tile_pool(name="psY", bufs=1, space="PSUM") as psY, \
             tc.tile_pool(name="zwork", bufs=3) as zw, \
             tc.tile_pool(name="ypool", bufs=4) as yp:
            # ---- emission helpers: stage B blocks + stage C group body,
            # interleaved so early C groups fill the PE while weights load ----
            state = {}
            def emit_kT():
                # kT (padded to HDP per head): [dout_block, t]
                for blk in range(8):
                    for ch in range(2):
                        p = psA.tile([128, 512], FP32, tag="pA", name="pK")
                        for ko in range(6):
                            nc.tensor.matmul(p[:], lhsT=wk_sb[:, ko, ts(blk, 128)],
                                             rhs=sT_sb[:, ko, ts(ch, 512)],
                                             start=(ko == 0), stop=(ko == 5))
                        nc.vector.tensor_copy(kT_sb[:, blk, ts(ch, 512)], p[:])

            def emit_v():
                # v (padded): [t_block, dout]
                for tb in range(8):
                    for ch in range(2):
                        p = psA.tile([128, 512], FP32, tag="pA", name="pV")
                        for ko in range(6):
                            nc.tensor.matmul(p[:], lhsT=sT_sb[:, ko, ts(tb, 128)],
                                             rhs=wv_sb[:, ko, ts(ch, 512)],
                                             start=(ko == 0), stop=(ko == 5))
                        nc.vector.tensor_copy(
                            v_sb[:, tb, ds(8 * ch, 8), 0:HDP],
                            p.rearrange("p (a b) -> p a b", a=8))

            def emit_qg():
                # qT for own rows, scaled by 1/sqrt(hd), bias added
                for blk in range(8):
                    p = psA.tile([128, 512], FP32, tag="pA", name="pQ")[:, :RPC]
                    for ko in range(6):
                        nc.tensor.matmul(p[:], lhsT=wq_sb[:, ko, ts(blk, 128)],
                                         rhs=sTc_sb[:, ko, :],
                                         start=(ko == 0), stop=(ko == 5))
                    nc.scalar.activation(qT_sb[:, blk, :], p[:], AF.Identity,
                                         bias=bq_sb[:, blk:blk + 1], scale=ISQ)
                # g for own rows
                for ch, w in ((0, 512), (1, 256)):
                    p = psA.tile([128, 512], FP32, tag="pA", name="pG")
                    for ko in range(6):
                        nc.tensor.matmul(p[:, :w], lhsT=sTc_sb[:, ko, :],
                                         rhs=wg_sb[:, ko, ds(512 * ch, w)],
                                         start=(ko == 0), stop=(ko == 5))
                    nc.vector.tensor_copy(g_sb[:, ds(512 * ch, w)], p[:, :w])

            # ---- stage C: pair-bias pipeline over own z rows ----
            # z comes pre-transposed from the host: one straight 1MB DMA per
            # 4-row group, alternating between the two HWDGE rings. Each row
            # j maps to PE col strip j; y (on zT) and ss (on zT^2) are
            # independent single-matmul groups into separate fp32 PSUM banks
            # (bank-wide has_written clear makes concurrent accumulation
            # chains in one bank unsafe), merged during evacuation.
            def emit_group(grp):
                zT4 = zw.tile([128, 4 * S], BF16, tag="zT4", bufs=3)
                eng = nc.sync if grp % 2 == 0 else nc.scalar
                eng.dma_start(zT4[:],
                              zbT[:, ds(4 * grp, 4), :].rearrange("c r t -> c (r t)"))
                sq4 = zw.tile([128, 4 * S], BF16, tag="sq4")
                nc.vector.tensor_tensor(sq4[:], zT4[:], zT4[:], ALU.mult)
                py = psY.tile([128, 2, 512], FP32, tag="py", name="py", bufs=2)
                pss = psY.tile([128, 2, 512], FP32, tag="pss", name="pss")
                for j in range(4):
                    for ch in range(2):
                        nc.tensor.matmul(py[ds(32 * j, 32), ch, :],
                                         lhsT=w18_sb[:],
                                         rhs=zT4[:, ds(S * j + 512 * ch, 512)],
                                         start=True, stop=True,
                                         tile_position=(0, 32 * j))
                for j in range(4):
                    for ch in range(2):
                        nc.tensor.matmul(pss[ds(32 * j, 32), ch, :],
                                         lhsT=wss_sb[:],
                                         rhs=sq4[:, ds(S * j + 512 * ch, 512)],
                                         start=True, stop=True,
                                         tile_position=(0, 32 * j))
                # Wss replicates E[z^2] into all 32 cols of each strip, so
                # both banks evacuate as full-width ACT copies (DVE cannot
                # read two PSUM operands; single-partition copies are
                # single-lane and ~20x slower). Evacuations land in 2-group
                # tiles; the SWDGE writes flush every other group at double
                # width to halve the per-instruction fixed cost.
                if grp % 2 == 0:
                    state["y4d"] = yp.tile([128, 2, 2, 512], BF16,
                                           tag="y4", bufs=3, name="y4d")
                    state["ss4d"] = yp.tile([128, 2, 2, 512], BF16,
                                            tag="ss4", bufs=2, name="ss4d")
                y4d, ss4d = state["y4d"], state["ss4d"]
                nc.scalar.copy(y4d[:, grp % 2, :, :], py[:])
                nc.scalar.copy(ss4d[:, grp % 2, :, :], pss[:])
                if grp % 2 == 1:
                    u = grp // 2
                    ydv = y_dram.rearrange("(u v) s n -> u v s n", v=4)
                    y4df = y4d.rearrange("p a b n -> p a (b n)")
                    for j in range(4):
                        nc.gpsimd.dma_start(
                            ydv[ds(2 * u, 2), j, ds(1, 17), :]
                            .rearrange("u s n -> s u n"),
                            y4df[ds(32 * j + 1, 17), :, :])
                    ss4dv = ss4d.rearrange("(j c) a b n -> j c a (b n)", c=32)
                    nc.gpsimd.dma_start(
                        ydv[ds(2 * u, 2), :, 0, :].rearrange("u v n -> v u n"),
                        ss4dv[:, 0, :, :])

            # interleaved emission: early C groups fill the PE while the
            # projection weights stream in, then alternate B blocks with C
            for grp in range(8):
                emit_group(grp)
            emit_kT()
            for grp in range(8, 12):
                emit_group(grp)
            emit_v()
            for grp in range(12, 16):
                emit_group(grp)
            emit_qg()
            for grp in range(16, RPC // 4):
                emit_group(grp)

        # ---- stage D: r and u from round-tripped stats ----
        nc.sync.dma_start(mu_sb[:], y_dram[:, 17, :])
        nc.sync.dma_start(ez2_sb[:], y_dram[:, 0, :])
        nc.vector.tensor_tensor(var_sb[:], mu_sb[:], mu_sb[:], ALU.mult)
        nc.vector.tensor_tensor(var_sb[:], ez2_sb[:], var_sb[:], ALU.subtract)
        nc.scalar.activation(var_sb[:], var_sb[:], AF.Ln, bias=eps_sb[:])
        nc.scalar.activation(r_sb[:], var_sb[:], AF.Exp, scale=-0.5)

        # ---- stage E: attention per head ----
        with tc.tile_pool(name="psE", bufs=2, space="PSUM") as psE, \
             tc.tile_pool(name="head", bufs=2) as hw_pool:
            for h in range(H):
                po2, blk = 64 * (h % 2), h // 2
                y_h = hw_pool.tile([128, S], BF16, tag="yh")
                if h % 2 == 0:
                    nc.sync.dma_start(y_h[:], y_dram[:, 1 + h, :])
                else:
                    nc.scalar.dma_start(y_h[:], y_dram[:, 1 + h, :])
                t1 = hw_pool.tile([128, S], BF16, tag="t1")
                nc.vector.tensor_tensor(t1[:], y_h[:], r_sb[:], ALU.mult)
                sc = hw_pool.tile([128, S], BF16, tag="sc")
                for ch in range(2):
                    pq = psE.tile([128, 512], FP32, tag="qk")
                    nc.tensor.matmul(pq[:],
                                     lhsT=qT_sb[ds(po2, 64), blk, :],
                                     rhs=kT_sb[ds(po2, 64), blk, ts(ch, 512)],
                                     start=True, stop=True)
                    nc.vector.tensor_tensor(sc[:, ts(ch, 512)], pq[:],
                                            t1[:, ts(ch, 512)], ALU.add)
                aT = hw_pool.tile([128, 8, 128], BF16, tag="aT")
                for half in range(2):
                    pt = psE.tile([128, 512], BF16, tag="pt")
                    for jj in range(4):
                        nc.tensor.transpose(pt[:, ts(jj, 128)],
                                            sc[:, ts(4 * half + jj, 128)],
                                            ident[:])
                    nc.scalar.activation(aT[:, ds(4 * half, 4), :],
                                         pt.rearrange("p (a b) -> p a b", a=4),
                                         AF.Exp)
                po = psE.tile([128, HDP + 1], FP32, tag="po")
                for tb in range(8):
                    nc.tensor.matmul(po[:], lhsT=aT[:, tb, :],
                                     rhs=v_sb[:, tb, h, :],
                                     start=(tb == 0), stop=(tb == 7))
                dr = hw_pool.tile([128, 1], FP32, tag="dr")
                nc.vector.reciprocal(dr[:], po[:, HDP:HDP + 1])
                nc.vector.tensor_scalar(oall[:, ds(HD * h, HD)], po[:, 0:HD],
                                        dr[:], None, op0=ALU.mult)

            # ---- stage F: gate + output projection ----
            sig = hw_pool.tile([128, D], BF16, tag="sig")
            nc.scalar.activation(sig[:], g_sb[:], AF.Sigmoid)
            og = hw_pool.tile([128, D], BF16, tag="og")
            nc.vector.tensor_tensor(og[:], oall[:], sig[:], ALU.mult)
            ogT = hw_pool.tile([128, 6, 128], BF16, tag="ogT")
            for half, n in ((0, 4), (1, 2)):
                pt = psE.tile([128, 512], BF16, tag="pt")
                for jj in range(n):
                    nc.tensor.transpose(pt[:, ts(jj, 128)],
                                        og[:, ts(4 * half + jj, 128)], ident[:])
                nc.vector.tensor_copy(
                    ogT[:, ds(4 * half, n), :],
                    pt.rearrange("p (a b) -> p a b", a=4)[:, 0:n, :])
            out_sb = hw_pool.tile([128, D], FP32, tag="outsb")
            for ch, w in ((0, 512), (1, 256)):
                pf = psE.tile([128, 512], FP32, tag="qk")
                for ko in range(6):
                    nc.tensor.matmul(pf[:, :w], lhsT=ogT[:, ko, :],
                                     rhs=wo_sb[:, ko, ds(512 * ch, w)],
                                     start=(ko == 0), stop=(ko == 5))
                nc.vector.tensor_copy(out_sb[:, ds(512 * ch, w)], pf[:, :w])
            nc.sync.dma_start(out[:], out_sb[:])

    nc.compile()
    return nc


def _prep(inputs):
    bf = ml_dtypes.bfloat16
    s = np.asarray(inputs["s"], np.float32)[0]
    z = np.asarray(inputs["z"], np.float32)[0]
    Wq = np.asarray(inputs["Wq"], np.float32)
    bq = np.asarray(inputs["bq"], np.float32)
    Wk = np.asarray(inputs["Wk"], np.float32)
    Wv = np.asarray(inputs["Wv"], np.float32)
    Wg = np.asarray(inputs["Wg"], np.float32)
    ln_w = np.asarray(inputs["ln_w"], np.float32)
    ln_b = np.asarray(inputs["ln_b"], np.float32)  # noqa: F841 (softmax-invariant)
    Wz = np.asarray(inputs["Wz"], np.float32)
    Wo = np.asarray(inputs["Wo"], np.float32)

    def pad_rows(W):
        Wp = np.zeros((DP, D), np.float32)
        for h in range(H):
            Wp[h * HDP:h * HDP + HD] = W[h * HD:(h + 1) * HD]
        return Wp

    zb = z.astype(bf)
    sT = np.ascontiguousarray(s.T).astype(bf)
    WqTp = np.ascontiguousarray(pad_rows(Wq).T).astype(bf)
    WkTp = np.ascontiguousarray(pad_rows(Wk).T).astype(bf)
    WvTp = np.ascontiguousarray(pad_rows(Wv).T).astype(bf)
    WgT = np.ascontiguousarray(Wg.T).astype(bf)
    WoT = np.ascontiguousarray(Wo.T).astype(bf)
    bq_p = np.zeros(DP, np.float32)
    for h in range(H):
        bq_p[h * HDP:h * HDP + HD] = bq[h * HD:(h + 1) * HD]
    bq_p *= ISQ

    Wzp = ln_w[None, :] * Wz                     # [H, DZ]
    # channel order per strip: 0 = E[z^2] (from Wss), 1..16 = heads, 17 = mean
    # head cols are pre-centered: y'_h = sum_c (w_c - c1_h/DZ) z_c
    # = y_h - c1_h*mu, so the bias is just r*y'_h (no mean subtraction)
    c1 = Wzp.sum(-1)
    W18 = np.zeros((DZ, 32), np.float32)
    W18[:, 1:1 + H] = Wzp.T - c1[None, :] / DZ
    W18[:, 17] = 1.0 / DZ
    Wss = np.full((DZ, 32), 1.0 / DZ, np.float32)
    c1 = [float(x) for x in c1]

    shared = {
        "sT": sT, "WqT": WqTp, "WkT": WkTp, "WvT": WvTp, "WgT": WgT,
        "WoT": WoT, "bqs": bq_p, "W18": W18.astype(bf), "Wss": Wss.astype(bf),
    }
    in_maps = []
    for ci in range(N_CORES):
        rows = slice(ci * RPC, (ci + 1) * RPC)
        m = dict(shared)
        # pre-transposed z slice: zbT[c, r, t] = z[r0+r, t, c]
        m["zbT"] = np.ascontiguousarray(zb[rows].transpose(2, 0, 1))
        m["sTc"] = np.ascontiguousarray(sT[:, rows])
        in_maps.append(m)
    return in_maps, c1


def _install_ntff_hook():
    try:
        import antenv
        from trn_agent_boot.trn_boot import _ntff_profile_via_ctypes
        from concourse import bass_utils
        mod = types.ModuleType("antenv.axon_hooks")
        mod._hook = _ntff_profile_via_ctypes('/opt/axon/libaxon_pjrt.so')
        mod.set_axon_ntff_profile_hook = lambda h: setattr(mod, "_hook", h)
        mod.get_axon_ntff_profile_hook = lambda: mod._hook
        sys.modules["antenv.axon_hooks"] = mod
        antenv.axon_hooks = mod
        bass_utils.upload_artifacts = lambda tmpdir: tmpdir
    except Exception as e:  # profiling is best-effort
        print(f"ntff hook install failed: {e}", file=sys.stderr)


def run(inputs, trace=False):
    from concourse.bass_utils import run_bass_kernel_spmd
    in_maps, c1 = _prep(inputs)
    key = tuple(np.round(c1, 6))
    if key not in _CACHE:
        _CACHE[key] = _build(c1)
    nc = _CACHE[key]
    if trace:
        _install_ntff_hook()
    res = run_bass_kernel_spmd(nc, in_maps, core_ids=list(range(N_CORES)),
                               trace=trace)
    out = np.concatenate([res.results[i]["out"] for i in range(N_CORES)], axis=0)
    return out[None].astype(np.float32), res


def kernel(**inputs) -> np.ndarray:
    out, _ = run(inputs, trace=bool(os.environ.get("KERNEL_TRACE")))
    return out

